# revision 13
# baseline (speedup 1.0000x reference)
"""GAT-style attention kernel for Trainium2, 8-core row-parallel.

Reference computation:
    h = x @ W; s1 = h @ a1; s2 = h @ a2
    e[i,j] = leaky_relu(s1[j] + s2[i], 0.2); masked by adj; row-softmax; @ h

Key algebraic trick: with the column rescale w~ = w / exp(0.2*s2[i]),
    w~[j,i] = adj[i,j] * max(exp(s1[j] + 0.8*s2[i]), exp(0.2*s1[j]))
and the rescale cancels in the softmax normalization:
    out[i,:] = (sum_j w~[j,i] h[j,:]) / (sum_j w~[j,i]).
So no separate leaky-relu pass is needed: one ACT Exp pass (per-partition
bias s1[j], scale 0.8 on the broadcast-s2 tile) plus one DVE
scalar_tensor_tensor (max with per-partition exp(0.2*s1[j]), then multiply
by the PE-transposed adj mask read straight from PSUM).

Per-core pipeline over j-chunks of 128 (i in blocks of 512):
    gpsimd cast-DMA: adj int32 -> bf16 slab [128p, RB, 512j]
    PE: 4x transpose [128i,128j] bf16 -> adjT psum [128j, 512i]
    ACT: ta = Exp(0.8*s2b + s1[jc])  [128, ROWS] f32
    DVE: wT = (ta max es1[jc]) * adjT -> bf16
    PE: out2[f,i] += h[jc] @ wT ; rowsum[1,i] += ones @ wT  (psum accum)
Finalize per i-block: reciprocal of rowsum, transpose back, scale, store.

Walrus codegen rejects instructions carrying more than one sync-wait
("Too many sync wait commands"), so after Tile scheduling we legalize the
program: excess waits are moved onto injected same-engine nop instructions
placed immediately before the over-constrained instruction.
"""

import copy
import sys
from contextlib import ExitStack

import numpy as np

if "/opt/trn_rl_repo" not in sys.path:
    sys.path.insert(0, "/opt/trn_rl_repo")

import concourse.bass as bass
import concourse.tile as tile
from concourse import mybir
from concourse.masks import make_identity

P = 128
N_CORES = 8

F32 = mybir.dt.float32
BF16 = mybir.dt.bfloat16
I32 = mybir.dt.int32
AX = mybir.AluOpType
AF = mybir.ActivationFunctionType

# Instruction types whose queue handles multi-wait natively (or that the
# framework emits and walrus already accepts).
_WAIT_SPLIT_SKIP = {"InstHalt", "InstSemWait", "InstEventSemOp"}


def _legalize_waits(nc, template_nop):
    """Move excess sync-waits onto injected same-engine nops."""
    uid = 0
    for f in nc.m.functions:
        for b in f.blocks:
            new_list = []
            changed = False
            for inst in b.instructions:
                si = inst.sync_info
                if (si is not None and len(si.on_wait) > 1
                        and type(inst).__name__ not in _WAIT_SPLIT_SKIP):
                    waits = list(si.on_wait)
                    for w in waits[:-1]:
                        uid += 1
                        nop = copy.copy(template_nop)
                        nop.name = f"I-lwsplit-{uid}"
                        nop.engine = inst.engine
                        nop.sync_info = mybir.SyncInfo(
                            on_wait=[w], on_update=[])
                        try:
                            nop.set_dependency_edges([])
                        except Exception:
                            pass
                        new_list.append(nop)
                    inst.sync_info = mybir.SyncInfo(
                        on_wait=[waits[-1]], on_update=list(si.on_update))
                    changed = True
                new_list.append(inst)
            if changed:
                b.instructions = new_list


def build_program(N=12288, IN_F=256, OUT_F=128, alpha=0.2, legalize=True):
    """Single-core SPMD program. Per-core inputs: adj_s [ROWS,N] i32,
    x [N,IN_F] f32 (full), xm [ROWS,IN_F] f32 (own rows), wx [IN_F,OUT_F]
    f32 (W), wa1/wa2 [1,IN_F] f32 (W@a1 / W@a2 rows). Output [ROWS,OUT_F].
    """
    ROWS = N // N_CORES
    NB = N // P
    KB = IN_F // P
    RB = ROWS // P
    IBS = 512 if ROWS % 512 == 0 else P
    IB = ROWS // IBS
    SUBS = IBS // P
    DJ = 512 if N % 512 == 0 else P
    JCC = N // DJ
    JPC = DJ // P

    nc = bass.Bass(trn_type="TRN2")
    adj_s = nc.dram_tensor("adj_s", [ROWS, N], I32, kind="ExternalInput")
    x_d = nc.dram_tensor("x", [N, IN_F], F32, kind="ExternalInput")
    xm_d = nc.dram_tensor("xm", [ROWS, IN_F], F32, kind="ExternalInput")
    wx_d = nc.dram_tensor("wx", [IN_F, OUT_F], F32, kind="ExternalInput")
    wa1_d = nc.dram_tensor("wa1", [1, IN_F], F32, kind="ExternalInput")
    wa2_d = nc.dram_tensor("wa2", [1, IN_F], F32, kind="ExternalInput")
    out_d = nc.dram_tensor("out", [ROWS, OUT_F], F32, kind="ExternalOutput")

    adj_v = adj_s[:, :].rearrange("(s p) j -> p s j", p=P)

    with tile.TileContext(nc) as tc, ExitStack() as ctx:
        template_nop = nc.sync.nop(nofuse=True).ins

        const = ctx.enter_context(tc.tile_pool(name="const", bufs=1))
        ident_f = const.tile([P, P], F32)
        make_identity(nc, ident_f[:])
        ident_b = const.tile([P, P], BF16)
        make_identity(nc, ident_b[:])
        ones_b = const.tile([P, 1], BF16)
        nc.gpsimd.memset(ones_b[:], 1.0)
        ones1_f = const.tile([1, P], F32)
        nc.gpsimd.memset(ones1_f[:], 1.0)

        h_sb = const.tile([P, NB * OUT_F], BF16)
        s1_sb = const.tile([P, NB], F32)
        es1_sb = const.tile([P, NB], F32)
        s2b = const.tile([P, ROWS], F32)
        wxb = const.tile([P, KB * OUT_F], BF16)
        wa2b = const.tile([P, IN_F], F32)
        wa1b = const.tile([P, IN_F], F32)

        # ---------------- setup ----------------
        with tc.tile_pool(name="su_ps", bufs=2, space="PSUM") as su_ps, \
             tc.tile_pool(name="su_sb", bufs=3) as su_sb:
            wxf = su_sb.tile([P, KB, OUT_F], F32, tag="wxf")
            nc.sync.dma_start(
                wxf[:], wx_d[:, :].rearrange("(c p) f -> p c f", p=P))
            nc.vector.tensor_copy(wxb[:], wxf[:].rearrange("p c f -> p (c f)"))

            nbc = [0]

            def bcast_row(dst, src_row, width):
                # dst[p, :width] = src_row[0, :width] for all 128 partitions
                for q in range(0, width, 512):
                    w = min(512, width - q)
                    ps = su_ps.tile([P, 512], F32, tag="bc",
                                    name=f"bc_{nbc[0]}")
                    nbc[0] += 1
                    nc.tensor.matmul(ps[:, :w], ones1_f[:],
                                     src_row[0:1, q:q + w],
                                     start=True, stop=True)
                    nc.scalar.copy(dst[:, q:q + w], ps[:, :w])

            wa2_sb = su_sb.tile([1, IN_F], F32, tag="wa2")
            nc.sync.dma_start(wa2_sb[:], wa2_d[:, :])
            bcast_row(wa2b, wa2_sb, IN_F)
            wa1_sb = su_sb.tile([1, IN_F], F32, tag="wa1")
            nc.sync.dma_start(wa1_sb[:], wa1_d[:, :])
            bcast_row(wa1b, wa1_sb, IN_F)

            # s2 of this core's rows (exact f32 on DVE)
            s2_loc = su_sb.tile([P, RB], F32, tag="s2loc")
            for rb in range(RB):
                xm_t = su_sb.tile([P, IN_F], F32, tag="xm")
                nc.sync.dma_start(xm_t[:], xm_d[rb * P:(rb + 1) * P, :])
                junk = su_sb.tile([P, IN_F], F32, tag="junk")
                nc.vector.scalar_tensor_tensor(
                    junk[:], xm_t[:], 1.0, wa2b[:], op0=AX.mult, op1=AX.mult,
                    accum_out=s2_loc[:, rb:rb + 1])
            s2T_ps = su_ps.tile([RB, P], F32, tag="trs")
            nc.tensor.transpose(s2T_ps[:], s2_loc[:], ident_f[:])
            s2T_sb = su_sb.tile([RB, P], F32, tag="trs_sb")
            nc.vector.tensor_copy(s2T_sb[:], s2T_ps[:])
            s2row = su_sb.tile([1, ROWS], F32, tag="s2row")
            nc.sync.dma_start(s2row[:], s2T_sb[:])
            bcast_row(s2b, s2row, ROWS)

            # h (bf16) and s1 (f32) per j-block
            for jb in range(NB):
                xb = su_sb.tile([P, IN_F], F32, tag="xb")
                nc.sync.dma_start(xb[:], x_d[jb * P:(jb + 1) * P, :])
                junk2 = su_sb.tile([P, IN_F], F32, tag="junk2")
                nc.vector.scalar_tensor_tensor(
                    junk2[:], xb[:], 1.0, wa1b[:], op0=AX.mult, op1=AX.mult,
                    accum_out=s1_sb[:, jb:jb + 1])
                xT_ps = su_ps.tile([P, IN_F], F32, tag="xT")
                for k2 in range(KB):
                    nc.tensor.transpose(
                        xT_ps[:, k2 * P:(k2 + 1) * P],
                        xb[:, k2 * P:(k2 + 1) * P], ident_f[:])
                xT_sb = su_sb.tile([P, KB, P], BF16, tag="xTs")
                nc.scalar.copy(xT_sb[:].rearrange("p c f -> p (c f)"), xT_ps[:])
                h_ps = su_ps.tile([P, OUT_F], F32, tag="h")
                for k2 in range(KB):
                    nc.tensor.matmul(
                        h_ps[:], xT_sb[:, k2, :],
                        wxb[:, k2 * OUT_F:(k2 + 1) * OUT_F],
                        start=(k2 == 0), stop=(k2 == KB - 1))
                nc.scalar.copy(h_sb[:, jb * OUT_F:(jb + 1) * OUT_F], h_ps[:])
            nc.scalar.activation(es1_sb[:], s1_sb[:], AF.Exp, scale=alpha)

        # ---------------- main loop ----------------
        ps_out = ctx.enter_context(tc.tile_pool(name="ps_out", bufs=1, space="PSUM"))
        ps_rs = ctx.enter_context(tc.tile_pool(name="ps_rs", bufs=1, space="PSUM"))
        ps_tr = ctx.enter_context(tc.tile_pool(name="ps_tr", bufs=2, space="PSUM"))
        adj_pool = ctx.enter_context(tc.tile_pool(name="adj", bufs=2))
        ta_pool = ctx.enter_context(tc.tile_pool(name="ta", bufs=4))
        wt_pool = ctx.enter_context(tc.tile_pool(name="wt", bufs=3))
        fin_pool = ctx.enter_context(tc.tile_pool(name="fin", bufs=2))

        out2 = [ps_out.tile([P, IBS], F32, tag=f"o{b}", name=f"out2_{b}")
                for b in range(IB)]
        rsum = [ps_rs.tile([1, IBS], F32, tag=f"r{b}", name=f"rsum_{b}")
                for b in range(IB)]

        for jcc in range(JCC):
            adj_bf = adj_pool.tile([P, RB, DJ], BF16)
            nc.gpsimd.dma_start(
                adj_bf[:], adj_v[:, :, jcc * DJ:(jcc + 1) * DJ])
            for js in range(JPC):
                jc = jcc * JPC + js
                first, last = jc == 0, jc == NB - 1
                ta = ta_pool.tile([P, ROWS], F32)
                nc.scalar.activation(
                    ta[:], s2b[:], AF.Exp,
                    bias=s1_sb[:, jc:jc + 1], scale=1.0 - alpha)
                wTs = []
                for b in range(IB):
                    adjT = ps_tr.tile([P, IBS], BF16, tag="tr")
                    for t in range(SUBS):
                        nc.tensor.transpose(
                            adjT[:, t * P:(t + 1) * P],
                            adj_bf[:, b * SUBS + t, js * P:(js + 1) * P],
                            ident_b[:])
                    wT = wt_pool.tile([P, IBS], BF16, name=f"wT_{jc}_{b}")
                    nc.vector.scalar_tensor_tensor(
                        wT[:], ta[:, b * IBS:(b + 1) * IBS],
                        es1_sb[:, jc:jc + 1], adjT[:],
                        op0=AX.max, op1=AX.mult)
                    wTs.append(wT)
                # keep the h slice stationary across all i-blocks, then the
                # ones vector for the rowsums (2 weight loads per chunk)
                for b in range(IB):
                    nc.tensor.matmul(
                        out2[b][:], h_sb[:, jc * OUT_F:(jc + 1) * OUT_F],
                        wTs[b][:], start=first, stop=last)
                for b in range(IB):
                    nc.tensor.matmul(rsum[b][:], ones_b[:], wTs[b][:],
                                     start=first, stop=last)

        # ---------------- finalize ----------------
        for b in range(IB):
            o_sb = fin_pool.tile([P, IBS], F32, tag="osb")
            nc.vector.tensor_copy(o_sb[:], out2[b][:])
            rs_sb = fin_pool.tile([1, IBS], F32, tag="rssb")
            nc.vector.tensor_copy(rs_sb[:], rsum[b][:])
            rall = fin_pool.tile([P, SUBS], F32, tag="rall")
            for t in range(SUBS):
                rT_ps = ps_tr.tile([P, 512], BF16, tag="tr", name=f"rT_{b}_{t}")
                rT = rT_ps[:, 0:2].bitcast(F32)
                nc.tensor.matmul(rT[:, 0:1], rs_sb[0:1, t * P:(t + 1) * P],
                                 ones1_f[0:1, 0:1], start=True, stop=True)
                nc.vector.tensor_copy(rall[:, t:t + 1], rT[:, 0:1])
            rinv = fin_pool.tile([P, SUBS], F32, tag="rinv")
            nc.vector.reciprocal(rinv[:], rall[:])
            for t in range(SUBS):
                oT_ps = ps_tr.tile([P, 512], BF16, tag="tr", name=f"oT_{b}_{t}")
                oT = oT_ps[:, 0:256].bitcast(F32)
                nc.tensor.transpose(oT[:], o_sb[:, t * P:(t + 1) * P],
                                    ident_f[:])
                fin = fin_pool.tile([P, OUT_F], F32, tag="fint")
                nc.vector.tensor_scalar_mul(fin[:], oT[:, :OUT_F],
                                            rinv[:, t:t + 1])
                nc.sync.dma_start(
                    out_d[b * IBS + t * P:b * IBS + (t + 1) * P, :], fin[:])

    if legalize:
        _legalize_waits(nc, template_nop)
    return nc


_PROG_CACHE = {}


def _get_program(N, IN_F, OUT_F):
    key = (N, IN_F, OUT_F)
    if key not in _PROG_CACHE:
        _PROG_CACHE[key] = build_program(N, IN_F, OUT_F)
    return _PROG_CACHE[key]


def make_in_maps(x, adj, W, a1, a2):
    N, IN_F = x.shape
    ROWS = N // N_CORES
    wx = np.ascontiguousarray(W, dtype=np.float32)
    wa1 = np.ascontiguousarray((W @ a1)[None, :], dtype=np.float32)
    wa2 = np.ascontiguousarray((W @ a2)[None, :], dtype=np.float32)
    in_maps = []
    for c in range(N_CORES):
        sl = slice(c * ROWS, (c + 1) * ROWS)
        in_maps.append({
            "adj_s": np.ascontiguousarray(adj[sl]),
            "x": np.ascontiguousarray(x),
            "xm": np.ascontiguousarray(x[sl]),
            "wx": wx,
            "wa1": wa1,
            "wa2": wa2,
        })
    return in_maps


def kernel(x, adj, W, a1, a2, trace=False):
    x = np.asarray(x, dtype=np.float32)
    adj = np.ascontiguousarray(np.asarray(adj, dtype=np.int32))
    W = np.asarray(W, dtype=np.float32)
    a1 = np.asarray(a1, dtype=np.float32)
    a2 = np.asarray(a2, dtype=np.float32)
    N, IN_F = x.shape
    OUT_F = W.shape[1]

    from concourse.bass_utils import run_bass_kernel_spmd

    nc = _get_program(N, IN_F, OUT_F)
    in_maps = make_in_maps(x, adj, W, a1, a2)
    res = run_bass_kernel_spmd(
        nc, in_maps, core_ids=list(range(N_CORES)), trace=trace)
    out = np.concatenate([r["out"] for r in res.results], axis=0)
    kernel.last_results = res
    return out
